# revision 20
# baseline (speedup 1.0000x reference)
"""CIF (Continuous Integrate-and-Fire) segment-reduce kernel for Trainium2 (8 NeuronCores).

Structure of the problem (B=32, T=2000, H=512, L_OUT=250, threshold=0.95):

  * The scan over T is a recurrence ONLY in the scalar integrator driven by
    `alphas` [B,T] (256 KB).  It never touches `hidden`.  We replicate the
    reference's sequential fp32 arithmetic exactly on the host (same op
    order -> bit-identical fire decisions), which yields, for every step t,
    at most two (output-slot, weight) contributions:
      - no fire:  alpha_t             -> slot n_prev
      - fire:     1 - integrate_{t-1} -> slot n_prev   (emitted frame's last term)
                  alpha_t - dist_comp -> slot n_prev+1 (next frame's first term)
    where n_prev = number of fires before t.  Contributions to slots that
    never get emitted (>= min(#fires, L_OUT)) are dropped, matching the
    reference's gather/valid masking.

  * The heavy part, out[b,l] = sum_t W[b,l,t] * hidden[b,t], is a banded
    matmul (band drift is exactly 15.625 slots per 125-step chunk since
    sum(alphas) == 250; deviation is a Brownian bridge, sigma <~2 slots).
    It runs on the 8 NeuronCores, data-parallel over B (4 examples/core).
    Per example the 250 output slots live in two PSUM "panels" (banks) of
    128 slots; each of the 16 T-chunks matmul-accumulates
    W_chunk[125,128]^T @ hidden_chunk[125,512] into the panel(s) its band
    intersects (chunks 0-9 -> panel 0, chunks 7-15 -> panel 1; the overlap
    chunks carry disjoint column halves of the band in each panel, which the
    weight builder asserts).  The vector engine then copies each panel to
    SBUF and the result is DMA'd out.

  * DMA strategy: hidden streams as per-chunk 256 KB DMAs (contiguous HBM
    reads) on the SWDGE (gpsimd) path — the only DGE whose queue spreads
    across all 16 SDMA engines (HWDGE rings only get 5) — casting
    fp32->fp16 in flight.  Weights ride the sync HWDGE ring, outputs the
    scalar HWDGE ring, all in parallel.  fp16 operands keep the PE on
    single-pass matmuls (fp32 is a 2-instruction LOW_HIGH decomposition,
    ~6x slower); the 2^-11 operand rounding costs ~3e-4 relative error.

Memory traffic per core ~ 16.4 MB hidden + 2.6 MB W + 2 MB out -> memory-bound.
"""

import numpy as np

B, T, H = 32, 2000, 512
L_OUT = 250
N_CORES = 8
EX_PER_CORE = B // N_CORES      # 4
NCHUNK = 16                     # T-chunks per example
KC = T // NCHUNK                # 125 steps per chunk
LPAD = 256                      # padded slot axis (2 panels x 128)

# Hidden streams in 3 blocks per example; partition p of a block tile holds
# the S consecutive timesteps t = t0 + S*p + j, j<S (one contiguous S*2 KB
# HBM read per partition -> large DMA descriptors -> full SDMA bandwidth).
# Each matmul contracts sub-chunk j = the 125 strided steps {t0 + S*p + j};
# the weight builder permutes W rows to match, so the sum is unchanged.
# Output slots live in two PSUM panels of 128.  Slot position at step t is
# t/8 +- dev (Brownian bridge, sigma ~1.6 slots), so block [0,875) can only
# touch panel 0 and block [1125,2000) only panel 1 (11+ sigma margins,
# asserted); the boundary block [875,1125) hits both.
BLOCKS = [  # (t0, t1, S = steps per partition line, panels)
    (0, 875, 7, (0,)),
    (875, 1125, 2, (0, 1)),
    (1125, 2000, 7, (1,)),
]
MMS = [
    (bl, j, p)
    for bl, (t0, t1, S, panels) in enumerate(BLOCKS)
    for p in panels
    for j in range(S)
]
NMM = len(MMS)                  # 18

_PROGRAM = None        # cached compiled Bass program
LAST_RESULT = None     # BassKernelResults of the most recent run (introspection)
RUN_KWARGS = {}        # extra kwargs for run_bass_kernel_spmd (e.g. trace=True)


def _host_scan_weights(alphas: np.ndarray):
    """Replicates the reference scan's fp32 arithmetic exactly.

    Returns (wa, Ai, wb, Bi, ntot): per-step primary weight/slot, secondary
    (fire-only) weight/slot, and total fires per row.
    """
    a = np.ascontiguousarray(alphas, dtype=np.float32)
    Bb, Tt = a.shape
    ONE = np.float32(1.0)
    TH = np.float32(0.95)
    integrate = np.zeros(Bb, np.float32)
    n = np.zeros(Bb, np.int32)
    wa = np.empty((Bb, Tt), np.float32)
    wb = np.zeros((Bb, Tt), np.float32)
    Ai = np.empty((Bb, Tt), np.int32)
    Bi = np.empty((Bb, Tt), np.int32)
    for t in range(Tt):
        al = a[:, t]
        dist = ONE - integrate          # distribution_completion (fp32)
        integ = integrate + al          # fp32, same single add as reference
        f = integ > TH
        cur = np.where(f, dist, al)
        wa[:, t] = cur
        Ai[:, t] = n                    # n_prev
        wb[:, t] = np.where(f, al - cur, np.float32(0.0))
        Bi[:, t] = n + 1
        n = n + f
        integrate = np.where(f, integ - ONE, integ)  # exact subtract (Sterbenz)
    return wa, Ai, wb, Bi, n


def _build_weight_windows(alphas: np.ndarray) -> np.ndarray:
    """Returns W [B, KC, NMM, 128] float16 panel weight tiles."""
    wa, Ai, wb, Bi, ntot = _host_scan_weights(alphas)
    lim = np.minimum(ntot, L_OUT)[:, None].astype(np.int32)
    wa = np.where(Ai < lim, wa, np.float32(0.0))
    wb = np.where(Bi < lim, wb, np.float32(0.0))

    Wd = np.zeros((B, T, LPAD), np.float32)
    bi = np.arange(B)[:, None]
    ti = np.arange(T)[None, :]
    Wd[bi, ti, np.minimum(Bi, LPAD - 1)] = wb
    Wd[bi, ti, np.minimum(Ai, LPAD - 1)] = wa

    # panel-coverage asserts: every block's band must be inside the union of
    # the panels it is assigned to.
    for bl, (t0, t1, S, panels) in enumerate(BLOCKS):
        if 0 not in panels and Wd[:, t0:t1, :128].any():
            raise AssertionError(f"block {bl} has panel-0 mass but no panel-0 matmul")
        if 1 not in panels and Wd[:, t0:t1, 128:].any():
            raise AssertionError(f"block {bl} has panel-1 mass but no panel-1 matmul")

    W = np.empty((B, KC, NMM, 128), np.float16)
    for i, (bl, j, p) in enumerate(MMS):
        t0, t1, S, _ = BLOCKS[bl]
        # [B, p(=partition), j, slot] with t = t0 + S*p + j
        blk = Wd[:, t0:t1, :].reshape(B, KC, S, LPAD)
        W[:, :, i, :] = blk[:, :, j, p * 128 : (p + 1) * 128]
    return np.ascontiguousarray(W)


def _build_program():
    """Builds + compiles the per-core Bass/Tile program (SPMD, shared)."""
    import concourse.bacc as bacc
    import concourse.mybir as mybir
    import concourse.tile as tile

    nc = bacc.Bacc("TRN2", target_bir_lowering=False, debug=False, num_devices=N_CORES)
    hid = nc.dram_tensor(
        "hidden_sh", [EX_PER_CORE, T, H], mybir.dt.float32, kind="ExternalInput"
    )
    wwin = nc.dram_tensor(
        "w_sh", [EX_PER_CORE, KC, NMM, 128], mybir.dt.float16, kind="ExternalInput"
    )
    out = nc.dram_tensor(
        "out_sh", [EX_PER_CORE, L_OUT, H], mybir.dt.float32, kind="ExternalOutput"
    )

    f32 = mybir.dt.float32
    f16 = mybir.dt.float16
    with tile.TileContext(nc) as tc:
        with (
            tc.tile_pool(name="hp0", bufs=EX_PER_CORE) as hpool0,
            tc.tile_pool(name="hp1", bufs=EX_PER_CORE) as hpool1,
            tc.tile_pool(name="hp2", bufs=EX_PER_CORE) as hpool2,
            tc.tile_pool(name="wp", bufs=EX_PER_CORE) as wpool,
            tc.tile_pool(name="ob", bufs=4) as opool,
            tc.tile_pool(name="psp", bufs=4, space="PSUM") as pspool,
        ):
            hpools = [hpool0, hpool1, hpool2]
            # emit all input DMAs up front (everything fits in SBUF at fp16):
            # hidden per-block on SWDGE (contiguous 14 KB reads per partition,
            # fp32->fp16 cast in flight), weights on the sync HWDGE ring.
            htiles = []
            wtiles = []
            for e in range(EX_PER_CORE):
                row = []
                for bl, (t0, t1, S, _) in enumerate(BLOCKS):
                    hsrc = hid[e, t0:t1, :].rearrange("(p j) h -> p j h", j=S)
                    ht = hpools[bl].tile([KC, S, H], f16, name=f"hb{bl}")
                    nc.gpsimd.dma_start(ht[:], hsrc)
                    row.append(ht)
                htiles.append(row)
                wt = wpool.tile([KC, NMM, 128], f16)
                nc.sync.dma_start(wt[:], wwin[e])
                wtiles.append(wt)

            for e in range(EX_PER_CORE):
                wt = wtiles[e]
                panels = [
                    pspool.tile([128, H], f32, name=f"panel{p}", tag=f"panel{p}")
                    for p in range(2)
                ]
                first = [True, True]
                last_i = {
                    p: max(i for i, m in enumerate(MMS) if m[2] == p) for p in (0, 1)
                }
                for i, (bl, j, p) in enumerate(MMS):
                    nc.tensor.matmul(
                        panels[p][:], wt[:, i, :], htiles[e][bl][:, j, :],
                        start=first[p], stop=(i == last_i[p]),
                    )
                    first[p] = False
                ob0 = opool.tile([128, H], f32)
                nc.vector.tensor_copy(ob0[:], panels[0][:])
                nc.scalar.dma_start(out[e, 0:128, :], ob0[:])
                ob1 = opool.tile([128, H], f32)
                nc.vector.tensor_copy(ob1[0 : L_OUT - 128, :], panels[1][0 : L_OUT - 128, :])
                nc.scalar.dma_start(out[e, 128:L_OUT, :], ob1[0 : L_OUT - 128, :])
    nc.compile()
    return nc


def kernel(hidden: np.ndarray, alphas: np.ndarray) -> np.ndarray:
    global _PROGRAM, LAST_RESULT
    from concourse.bass_utils import run_bass_kernel_spmd

    hidden = np.ascontiguousarray(np.asarray(hidden), dtype=np.float32)
    alphas = np.ascontiguousarray(np.asarray(alphas), dtype=np.float32)
    assert hidden.shape == (B, T, H) and alphas.shape == (B, T)

    Wwin = _build_weight_windows(alphas)

    if _PROGRAM is None:
        _PROGRAM = _build_program()
    nc = _PROGRAM

    in_maps = [
        {
            "hidden_sh": hidden[i * EX_PER_CORE : (i + 1) * EX_PER_CORE],
            "w_sh": Wwin[i * EX_PER_CORE : (i + 1) * EX_PER_CORE],
        }
        for i in range(N_CORES)
    ]
    res = run_bass_kernel_spmd(nc, in_maps, list(range(N_CORES)), **RUN_KWARGS)
    LAST_RESULT = res
    return np.concatenate([r["out_sh"] for r in res.results], axis=0)
